# revision 8
# baseline (speedup 1.0000x reference)
"""Trainium2 Bass kernel for nn_MinkUNetClassifier (UNet classifier, 4x512x512x1 -> 4x1).

Sharding: 8 cores = 4 images x 2 H-halves. Each core gets a 352-row slab
(256 owned rows + 48-row halo each side, zero-filled outside the image) and
computes the whole UNet on its slab with per-layer shrinking row windows
(halo recompute). BatchNorm statistics are exact: per-core partial sums are
AllReduce'd across the 8 cores at each of the 15 BN sites.

Layout conventions:
- DRAM activation tensors are [C, R, W+2] fp32 with one zero pad column on
  each side (zeroed once at kernel start); rows r0..r1 at that UNet level.
- SBUF conv input tiles are [128, rows, W+2]; channels live on partition
  blocks of 32/64/128 ("chunks", one per spatial sub-slab) so small-K
  matmuls run 4/2-way concurrent via tile_position row/col groups.
- The owned region is split into equal chunks (uniform slots: one shared
  PSUM bank, one evacuation ACT, one bn_stats per step across all chunks);
  halo rows are distributed across chunks and emitted per-chunk.
"""

import numpy as np

# ---------------------------------------------------------------- geometry
B, H, WIDTH = 4, 512, 512
SLAB = 352            # rows per core slab at full res (256 owned + 2*48 halo)
HALO = 48
LVL_W = [512, 256, 128, 64]
LVL_OWN = [(48, 304), (24, 152), (12, 76), (6, 38)]
NPX = [B * H * WIDTH // (4 ** l) for l in range(4)]  # global px per channel per level
RPC = 8               # output rows per band per chunk

# name -> (C, r0, r1, lvl)
TENS = {
    'h0':   (16, 2, 350, 0),
    'e0y1': (32, 3, 349, 0), 'e0y2': (32, 4, 348, 0), 'e0o': (32, 4, 348, 0),
    'd0':   (32, 2, 174, 1),
    'e1y1': (64, 3, 173, 1), 'e1y2': (64, 4, 172, 1), 'e1o': (64, 4, 172, 1),
    'd1':   (64, 2, 86, 2),
    'e2y1': (128, 3, 85, 2), 'e2y2': (128, 4, 84, 2), 'e2o': (128, 4, 84, 2),
    'd2':   (128, 2, 42, 3),
    'by1':  (128, 3, 41, 3), 'by2': (128, 4, 40, 3), 'bo': (128, 4, 40, 3),
    'u0':   (64, 8, 80, 2),
    'f0y1': (64, 9, 79, 2), 'f0y2': (64, 10, 78, 2), 'f0o': (64, 10, 78, 2),
    'u1':   (32, 21, 155, 1),
    'f1y1': (32, 22, 154, 1), 'f1y2': (32, 23, 153, 1), 'f1o': (32, 23, 153, 1),
    'u2':   (16, 46, 306, 0),
    'f2y1': (16, 47, 305, 0), 'f2y2': (16, 48, 304, 0), 'f2o': (16, 48, 304, 0),
}

# BN sites: idx -> (C, lvl, prod (nch, blk), cons (nch, blk), param path)
BNS = [
    (32, 0, (4, 32), (4, 32), ('enc', 0, 'g1')),    # 0  e0y1
    (32, 0, (4, 32), (4, 32), ('enc', 0, 'g2')),    # 1  e0y2
    (64, 1, (2, 64), (2, 64), ('enc', 1, 'g1')),    # 2  e1y1
    (64, 1, (2, 64), (2, 64), ('enc', 1, 'g2')),    # 3  e1y2
    (128, 2, (1, 128), (1, 128), ('enc', 2, 'g1')),  # 4  e2y1
    (128, 2, (1, 128), (1, 128), ('enc', 2, 'g2')),  # 5  e2y2
    (128, 3, (1, 128), (1, 128), ('bott', None, 'g1')),  # 6 by1
    (128, 3, (1, 128), (1, 128), ('bott', None, 'g2')),  # 7 by2
    (64, 2, (1, 64), (2, 64), ('dec', 0, 'g1')),    # 8  f0y1
    (64, 2, (2, 64), (1, 64), ('dec', 0, 'g2')),    # 9  f0y2
    (32, 1, (1, 32), (4, 32), ('dec', 1, 'g1')),    # 10 f1y1
    (32, 1, (4, 32), (1, 32), ('dec', 1, 'g2')),    # 11 f1y2
    (16, 0, (2, 64), (4, 32), ('dec', 2, 'g1')),    # 12 f2y1
    (16, 0, (4, 32), (2, 64), ('dec', 2, 'g2')),    # 13 f2y2
    (16, 0, (2, 64), (4, 32), ('out', None, 'g')),  # 14 f2o (final bn)
]
EPS = 1e-5

PASSES = [
    dict(kind='conv0', name='conv0', out='h0', nch=4, blk=32),
    dict(kind='conv', name='e0c1', srcs=[('h0', 0, 16, 0, 0)], prep=None, out='e0y1',
         wkey=('enc', 0, 'w1'), nch=4, blk=32, stats=0),
    dict(kind='conv', name='e0c2', srcs=[('e0y1', 0, 32, 0, 0)], prep=0, out='e0y2',
         wkey=('enc', 0, 'w2'), nch=4, blk=32, stats=1),
    dict(kind='comb', name='e0cb', y2='e0y2', ybn=1, srcs=[('h0', 0, 16, 0, 0)],
         wkey=('enc', 0, 'ws'), out='e0o', nch=4, blk=32, stats=None),
    dict(kind='down', name='e0dn', srcs=[('e0o', 0, 32, 0, 0)], out='d0',
         wkey=('enc', 0, 'down'), nch=4, blk=32),
    dict(kind='conv', name='e1c1', srcs=[('d0', 0, 32, 0, 0)], prep=None, out='e1y1',
         wkey=('enc', 1, 'w1'), nch=2, blk=64, stats=2),
    dict(kind='conv', name='e1c2', srcs=[('e1y1', 0, 64, 0, 0)], prep=2, out='e1y2',
         wkey=('enc', 1, 'w2'), nch=2, blk=64, stats=3),
    dict(kind='comb', name='e1cb', y2='e1y2', ybn=3, srcs=[('d0', 0, 32, 0, 0)],
         wkey=('enc', 1, 'ws'), out='e1o', nch=2, blk=64, stats=None),
    dict(kind='down', name='e1dn', srcs=[('e1o', 0, 64, 0, 0)], out='d1',
         wkey=('enc', 1, 'down'), nch=2, blk=64),
    dict(kind='conv', name='e2c1', srcs=[('d1', 0, 64, 0, 0)], prep=None, out='e2y1',
         wkey=('enc', 2, 'w1'), nch=1, blk=128, stats=4),
    dict(kind='conv', name='e2c2', srcs=[('e2y1', 0, 128, 0, 0)], prep=4, out='e2y2',
         wkey=('enc', 2, 'w2'), nch=1, blk=128, stats=5),
    dict(kind='comb', name='e2cb', y2='e2y2', ybn=5, srcs=[('d1', 0, 64, 0, 0)],
         wkey=('enc', 2, 'ws'), out='e2o', nch=1, blk=128, stats=None),
    dict(kind='down', name='e2dn', srcs=[('e2o', 0, 128, 0, 0)], out='d2',
         wkey=('enc', 2, 'down'), nch=1, blk=128),
    dict(kind='conv', name='bc1', srcs=[('d2', 0, 128, 0, 0)], prep=None, out='by1',
         wkey=('bott', None, 'w1'), nch=1, blk=128, stats=6, rps=2),
    dict(kind='conv', name='bc2', srcs=[('by1', 0, 128, 0, 0)], prep=6, out='by2',
         wkey=('bott', None, 'w2'), nch=1, blk=128, stats=7, rps=2),
    dict(kind='comb', name='bcb', y2='by2', ybn=7, srcs=[('d2', 0, 128, 0, 0)],
         wkey=None, out='bo', nch=1, blk=128, stats=None),
    dict(kind='up', name='up0', srcs=[('bo', 0, 128, 0, 0)], out='u0',
         wkey=('dec', 0, 'up'), nch=1, blk=128),
    dict(kind='conv', name='f0c1', srcs=[('u0', 0, 64, 0, 0), ('e2o', 0, 64, 0, 64), ('e2o', 64, 128, 1, 0)],
         prep=None, out='f0y1', wkey=('dec', 0, 'w1'), nch=1, blk=128, stats=8),
    dict(kind='conv', name='f0c2', srcs=[('f0y1', 0, 64, 0, 0)], prep=8, out='f0y2',
         wkey=('dec', 0, 'w2'), nch=2, blk=64, stats=9),
    dict(kind='comb', name='f0cb', y2='f0y2', ybn=9, srcs=[('u0', 0, 64, 0, 0), ('e2o', 0, 64, 0, 64), ('e2o', 64, 128, 1, 0)],
         wkey=('dec', 0, 'ws'), out='f0o', nch=1, blk=128, stats=None),
    dict(kind='up', name='up1', srcs=[('f0o', 0, 64, 0, 0)], out='u1',
         wkey=('dec', 1, 'up'), nch=2, blk=64),
    dict(kind='conv', name='f1c1', srcs=[('u1', 0, 32, 0, 0), ('e1o', 0, 64, 0, 32)],
         prep=None, out='f1y1', wkey=('dec', 1, 'w1'), nch=1, blk=128, stats=10),
    dict(kind='conv', name='f1c2', srcs=[('f1y1', 0, 32, 0, 0)], prep=10, out='f1y2',
         wkey=('dec', 1, 'w2'), nch=4, blk=32, stats=11),
    dict(kind='comb', name='f1cb', y2='f1y2', ybn=11, srcs=[('u1', 0, 32, 0, 0), ('e1o', 0, 64, 0, 32)],
         wkey=('dec', 1, 'ws'), out='f1o', nch=1, blk=128, stats=None),
    dict(kind='up', name='up2', srcs=[('f1o', 0, 32, 0, 0)], out='u2',
         wkey=('dec', 2, 'up'), nch=4, blk=32),
    dict(kind='conv', name='f2c1', srcs=[('u2', 0, 16, 0, 0), ('e0o', 0, 32, 0, 16)],
         prep=None, out='f2y1', wkey=('dec', 2, 'w1'), nch=2, blk=64, stats=12),
    dict(kind='conv', name='f2c2', srcs=[('f2y1', 0, 16, 0, 0)], prep=12, out='f2y2',
         wkey=('dec', 2, 'w2'), nch=4, blk=32, stats=13),
    dict(kind='comb', name='f2cb', y2='f2y2', ybn=13, srcs=[('u2', 0, 16, 0, 0), ('e0o', 0, 32, 0, 16)],
         wkey=('dec', 2, 'ws'), out='f2o', nch=2, blk=64, stats=14),
    dict(kind='pool', name='pool', srcs=[('f2o', 0, 16, 0, 0)], prep=14, nch=4, blk=32),
]


def _wget(params, wkey):
    grp, idx, leaf = wkey
    if grp == 'enc':
        node = params['enc'][idx]
        return node['down'] if leaf == 'down' else node['res'][leaf]
    if grp == 'dec':
        node = params['dec'][idx]
        return node['up'] if leaf == 'up' else node['res'][leaf]
    if grp == 'bott':
        return params['bott'][leaf]
    raise KeyError(wkey)


def _bn_params(params, bni):
    grp, idx, leaf = BNS[bni][4]
    if grp == 'enc':
        n = params['enc'][idx]['res']
    elif grp == 'dec':
        n = params['dec'][idx]['res']
    elif grp == 'bott':
        n = params['bott']
    else:
        return params['g_out'], params['b_out']
    return (n['g1'], n['b1']) if leaf == 'g1' else (n['g2'], n['b2'])


# ------------------------------------------------------------ weight plan
def build_wplan():
    """plan[pass] = list over taps of list of pieces (tile_idx, poff, kn, krow0, col, cout)."""
    plan = {}
    col = 0
    for P in PASSES:
        k = P['kind']
        if k == 'pool':
            continue
        if k == 'conv0':
            plan[P['name']] = [[(0, 0, 9, 0, col, 16)]]
            col += 16
            continue
        if k == 'comb' and P['wkey'] is None:
            plan[P['name']] = []
            continue
        cout = TENS[P['out']][0]
        ntap = {'conv': 9, 'comb': 1, 'down': 4, 'up': 4}[k]
        pieces = []
        kcum = 0
        cur = None
        for (tname, c0, c1, tix, poff) in P['srcs']:
            n = c1 - c0
            if cur is not None and cur[0] == tix and cur[1] + cur[2] == poff:
                cur = (cur[0], cur[1], cur[2] + n, cur[3])
            else:
                if cur is not None:
                    pieces.append(cur)
                cur = (tix, poff, n, kcum)
            kcum += n
        pieces.append(cur)
        taps = []
        for t in range(ntap):
            tp = []
            for (tix, poff, kn, krow0) in pieces:
                tp.append((tix, poff, kn, krow0, col, cout))
                col += cout
            taps.append(tp)
        plan[P['name']] = taps
    return plan, col


WPLAN, NCOL = build_wplan()


def pack_weights(params):
    wpack = np.zeros((128, NCOL), np.float32)
    for P in PASSES:
        k = P['kind']
        if k == 'pool' or (k == 'comb' and P['wkey'] is None):
            continue
        nch, blk = P['nch'], P['blk']
        if k == 'conv0':
            wm = np.asarray(params['conv0'], np.float32).reshape(9, 16)
            (tix, poff, kn, krow0, col, cout) = WPLAN['conv0'][0][0]
            for g in range(nch):
                wpack[blk * g: blk * g + 9, col: col + 16] = wm
            continue
        w = np.asarray(_wget(params, P['wkey']), np.float32)
        if k == 'up':
            w = w[::-1, ::-1]  # conv_transpose uses the flipped kernel
        if k == 'conv':
            tapmats = [w[dy, dx] for dy in range(3) for dx in range(3)]
        elif k in ('down', 'up'):
            tapmats = [w[a, b] for a in range(2) for b in range(2)]
        else:
            tapmats = [w[0, 0]]
        for t, wm in enumerate(tapmats):
            for (tix, poff, kn, krow0, col, cout) in WPLAN[P['name']][t]:
                sub = wm[krow0: krow0 + kn, :]
                for g in range(nch):
                    wpack[g * blk + poff: g * blk + poff + kn, col: col + cout] = sub
    return wpack


def pack_bn(params):
    """bnpack [128, nbn, 3]: g (cons layout), b (cons layout), owned-count (prod layout)."""
    bnpack = np.zeros((128, len(BNS), 3), np.float32)
    for i, (C, lvl, prod, cons, _) in enumerate(BNS):
        g, b = _bn_params(params, i)
        g = np.asarray(g, np.float32)
        b = np.asarray(b, np.float32)
        ncc, cblk = cons
        for q in range(ncc):
            bnpack[q * cblk: q * cblk + C, i, 0] = g
            bnpack[q * cblk: q * cblk + C, i, 1] = b
        npr, pblk = prod
        O0, O1 = LVL_OWN[lvl]
        n = (O1 - O0) // npr * LVL_W[lvl]
        for q in range(npr):
            bnpack[q * pblk: q * pblk + C, i, 2] = float(n)
    return bnpack


def slot_plan(a, b, O0, O1, nch, rpc):
    """Owned slots (uniform across chunks) + halo slots (per chunk or None)."""
    S = (O1 - O0) // nch
    assert S % rpc == 0, (a, b, O0, O1, nch, rpc)
    owned = [[(O0 + g * S + i * rpc, O0 + g * S + (i + 1) * rpc) for g in range(nch)]
             for i in range(S // rpc)]
    if nch == 1:
        per = [[(a, O0), (O1, b)]]
    elif nch == 2:
        per = [[(a, O0)], [(O1, b)]]
    else:
        fm, bm = (a + O0 + 1) // 2, (O1 + b) // 2
        per = [[(a, fm)], [(fm, O0)], [(O1, bm)], [(bm, b)]]
    halo_bands = []
    for rng in per:
        bands = []
        for (s, e) in rng:
            for r in range(s, e, rpc):
                bands.append((r, min(r + rpc, e)))
        halo_bands.append(bands)
    nhb = max(len(x) for x in halo_bands) if halo_bands else 0
    halo = [[halo_bands[g][i] if i < len(halo_bands[g]) else None for g in range(nch)]
            for i in range(nhb)]
    return owned, halo


# ------------------------------------------------------------ bass builder
def build_bass():
    import concourse.bass as bass
    import concourse.tile as tile
    from concourse import bacc, mybir

    F32 = mybir.dt.float32
    AF = mybir.ActivationFunctionType
    ALU = mybir.AluOpType

    nc = bacc.Bacc("TRN2", target_bir_lowering=False, debug=False,
                   enable_asserts=False, num_devices=8)

    x_in = nc.dram_tensor("x", [SLAB, 514], F32, kind="ExternalInput")
    w_in = nc.dram_tensor("w", [128, NCOL], F32, kind="ExternalInput")
    bn_in = nc.dram_tensor("bn", [128, len(BNS), 3], F32, kind="ExternalInput")
    pooled = nc.dram_tensor("pooled", [128, 1], F32, kind="ExternalOutput")

    dts = {}
    for name, (C, r0, r1, lvl) in TENS.items():
        dts[name] = nc.dram_tensor(name, [C, r1 - r0, LVL_W[lvl] + 2], F32, kind="Internal")
    stat_a, stat_b = {}, {}
    for i, (C, lvl, prod, cons, _) in enumerate(BNS):
        stat_a[i] = nc.dram_tensor(f"sa{i}", [prod[0] * C * 2], F32, kind="Internal")
        stat_b[i] = nc.dram_tensor(f"sb{i}", [prod[0] * C * 2], F32, kind="Internal",
                                   addr_space="Shared")

    with tile.TileContext(nc) as tc:
        with (
            tc.tile_pool(name="singles", bufs=1) as singles,
            tc.tile_pool(name="inb", bufs=2) as inb,
            tc.tile_pool(name="outb", bufs=2) as outb,
            tc.tile_pool(name="ps", bufs=6, space="PSUM") as psp,
            tc.tile_pool(name="stats", bufs=2) as statsp,
            tc.tile_pool(name="misc", bufs=3) as misc,
        ):
            wsb = singles.tile([128, NCOL], F32)
            nc.sync.dma_start(out=wsb, in_=w_in[:, :])
            bnp = singles.tile([128, len(BNS), 3], F32)
            nc.sync.dma_start(out=bnp, in_=bn_in[:, :, :])
            epst = singles.tile([128, 1], F32)
            nc.vector.memset(epst, EPS)
            abn = singles.tile([128, len(BNS), 2], F32)
            zt = singles.tile([128, 1024], F32)
            nc.vector.memset(zt, 0.0)

            for name, (C, r0, r1, lvl) in TENS.items():
                R = r1 - r0
                WP = LVL_W[lvl] + 2
                dst = bass.AP(tensor=dts[name], offset=0,
                              ap=[[R * WP, C], [WP, R], [WP - 1, 2]])
                nc.sync.dma_start(out=dst, in_=zt[0:C, 0: 2 * R])

            def dram_ap(tname, ch0, ch1, ra, rb, interior=False):
                C, r0, r1, lvl = TENS[tname]
                W = LVL_W[lvl]
                assert r0 <= ra <= rb <= r1, (tname, ra, rb, r0, r1)
                if interior:
                    return dts[tname][ch0:ch1, ra - r0: rb - r0, 1: W + 1]
                return dts[tname][ch0:ch1, ra - r0: rb - r0, :]

            def load_band(P, tiles, g, ra, rb, halo):
                blk = P['blk']
                for (tname, c0, c1, tix, poff) in P['srcs']:
                    nc.sync.dma_start(
                        out=tiles[tix][g * blk + poff: g * blk + poff + (c1 - c0),
                                       0: rb - ra + 2 * halo, :],
                        in_=dram_ap(tname, c0, c1, ra - halo, rb + halo))

            def emit_stats_reduce(sid, stile, nslots):
                C, lvl, (npr, pblk), (ncc, cblk), _ = BNS[sid]
                mv = misc.tile([128, 2], F32, tag="mv")
                nc.vector.bn_aggr(out=mv, in_=stile[:, 0:nslots, :])
                sums = misc.tile([128, 2], F32, tag="sums")
                nc.vector.tensor_tensor(out=sums[:, 0:1], in0=mv[:, 0:1], in1=mv[:, 0:1], op=ALU.mult)
                nc.vector.tensor_tensor(out=sums[:, 1:2], in0=mv[:, 1:2], in1=sums[:, 0:1], op=ALU.add)
                nc.vector.tensor_tensor(out=sums[:, 1:2], in0=sums[:, 1:2], in1=bnp[:, sid, 2:3], op=ALU.mult)
                nc.vector.tensor_tensor(out=sums[:, 0:1], in0=mv[:, 0:1], in1=bnp[:, sid, 2:3], op=ALU.mult)
                for q in range(npr):
                    dst = bass.AP(tensor=stat_a[sid], offset=q * C * 2, ap=[[2, C], [1, 2]])
                    nc.sync.dma_start(out=dst, in_=sums[q * pblk: q * pblk + C, :])
                nc.gpsimd.collective_compute(
                    "AllReduce", ALU.add, replica_groups=[list(range(8))],
                    ins=[stat_a[sid][:]], outs=[stat_b[sid][:]])
                gsq = misc.tile([128, max(npr, 1), 2], F32, tag="gsq")
                for q in range(ncc):
                    src = bass.AP(tensor=stat_b[sid], offset=0,
                                  ap=[[2, C], [C * 2, npr], [1, 2]])
                    nc.sync.dma_start(out=gsq[q * cblk: q * cblk + C, :, :], in_=src)
                SQ = misc.tile([128, 2], F32, tag="SQ")
                nc.vector.tensor_reduce(out=SQ, in_=gsq[:, :, :].rearrange("p n two -> p two n"),
                                        axis=mybir.AxisListType.X, op=ALU.add)
                ninv = 1.0 / float(NPX[lvl])
                m = misc.tile([128, 1], F32, tag="m")
                v = misc.tile([128, 1], F32, tag="v")
                nc.vector.tensor_scalar_mul(out=m, in0=SQ[:, 0:1], scalar1=ninv)
                nc.vector.tensor_scalar_mul(out=v, in0=SQ[:, 1:2], scalar1=ninv)
                t0 = misc.tile([128, 1], F32, tag="t0")
                nc.vector.tensor_tensor(out=t0, in0=m, in1=m, op=ALU.mult)
                nc.vector.tensor_tensor(out=v, in0=v, in1=t0, op=ALU.subtract)
                nc.scalar.activation(out=v, in_=v, func=AF.Sqrt, bias=epst, scale=1.0)
                nc.vector.reciprocal(out=v, in_=v)
                nc.vector.tensor_tensor(out=abn[:, sid, 0:1], in0=v, in1=bnp[:, sid, 0:1], op=ALU.mult)
                nc.vector.tensor_tensor(out=t0, in0=m, in1=abn[:, sid, 0:1], op=ALU.mult)
                nc.vector.tensor_tensor(out=abn[:, sid, 1:2], in0=bnp[:, sid, 1:2], in1=t0, op=ALU.subtract)

            # --------------------------------------------------------- conv
            def conv_chunk_mms(P, taps, tiles, pt, g, j, nr, W, Cout, dy_dx_of):
                """All tap matmuls for chunk g, step rows [j, j+nr) of its band."""
                blk = P['blk']
                last_tix = P['srcs'][-1][3]
                first = True
                ntap = len(taps)
                for t, tp in enumerate(taps):
                    dy, dx = dy_dx_of(t)
                    for (tix, poff, kn, krow0, col, co) in tp:
                        nc.tensor.matmul(
                            out=pt[g * blk: g * blk + Cout, 0: nr * W],
                            lhsT=wsb[g * blk + poff: g * blk + poff + kn, col: col + co],
                            rhs=tiles[tix][g * blk + poff: g * blk + poff + kn,
                                           j + dy: j + dy + nr, dx: dx + W],
                            start=first,
                            stop=(t == ntap - 1 and tix == last_tix),
                            tile_position=(g * blk, g * blk) if P['nch'] > 1 else None)
                        first = False

            def emit_conv(P):
                name = P['name']
                Cout, a, b, lvl = TENS[P['out']]
                W = LVL_W[lvl]
                WP = W + 2
                O0, O1 = LVL_OWN[lvl]
                nch, blk = P['nch'], P['blk']
                rps = P.get('rps', max(1, 512 // W))
                owned, halo = slot_plan(a, b, O0, O1, nch, RPC)
                sid = P.get('stats')
                taps = WPLAN[name]
                two = any(s[3] == 1 for s in P['srcs'])
                stile = None
                scnt = 0
                if sid is not None:
                    nslots = (LVL_OWN[lvl][1] - LVL_OWN[lvl][0]) // nch // rps
                    stile = statsp.tile([128, nslots, 6], F32, tag="st")

                def tap_of(t):
                    return (t // 3, t % 3)

                # owned slots: uniform
                for chunks in owned:
                    xt0 = inb.tile([128, RPC + 2, WP], F32, tag="xin")
                    tiles = [xt0]
                    if two:
                        xt1 = inb.tile([128, RPC + 2, WP], F32, tag="xin2")
                        tiles.append(xt1)
                    for g, (r0, r1) in enumerate(chunks):
                        load_band(P, tiles, g, r0, r1, 1)
                    if P['prep'] is not None:
                        nc.scalar.activation(
                            out=xt0[:, :, 1: W + 1], in_=xt0[:, :, 1: W + 1],
                            func=AF.Relu, scale=abn[:, P['prep'], 0:1],
                            bias=abn[:, P['prep'], 1:2])
                    ot = outb.tile([128, RPC, W], F32, tag="ot")
                    for j in range(0, RPC, rps):
                        pt = psp.tile([128, 512], F32, tag="pt")
                        for g in range(nch):
                            conv_chunk_mms(P, taps, tiles, pt, g, j, rps, W, Cout, tap_of)
                        nc.scalar.copy(out=ot[:, j: j + rps, :], in_=pt[:, 0: rps * W])
                        if sid is not None:
                            nc.vector.bn_stats(out=stile[:, scnt, :],
                                               in_=pt[:, 0: rps * W])
                            scnt += 1
                    for g, (r0, r1) in enumerate(chunks):
                        nc.sync.dma_start(
                            out=dram_ap(P['out'], 0, Cout, r0, r1, interior=True),
                            in_=ot[g * blk: g * blk + Cout, :, 0:W])
                # halo slots: per chunk
                for chunks in halo:
                    xt0 = inb.tile([128, RPC + 2, WP], F32, tag="xin")
                    tiles = [xt0]
                    if two:
                        xt1 = inb.tile([128, RPC + 2, WP], F32, tag="xin2")
                        tiles.append(xt1)
                    ot = outb.tile([128, RPC, W], F32, tag="ot")
                    for g, rr in enumerate(chunks):
                        if rr is None:
                            continue
                        r0, r1 = rr
                        nr = r1 - r0
                        load_band(P, tiles, g, r0, r1, 1)
                        if P['prep'] is not None:
                            nc.scalar.activation(
                                out=xt0[g * blk: (g + 1) * blk, 0: nr + 2, 1: W + 1],
                                in_=xt0[g * blk: (g + 1) * blk, 0: nr + 2, 1: W + 1],
                                func=AF.Relu, scale=abn[g * blk: (g + 1) * blk, P['prep'], 0:1],
                                bias=abn[g * blk: (g + 1) * blk, P['prep'], 1:2])
                        j = 0
                        while j < nr:
                            nrs = min(rps, nr - j)
                            pt = psp.tile([128, 512], F32, tag="pt")
                            conv_chunk_mms(P, taps, tiles, pt, g, j, nrs, W, Cout, tap_of)
                            nc.scalar.copy(out=ot[g * blk: g * blk + Cout, j: j + nrs, :],
                                           in_=pt[g * blk: g * blk + Cout, 0: nrs * W])
                            j += nrs
                        nc.sync.dma_start(
                            out=dram_ap(P['out'], 0, Cout, r0, r1, interior=True),
                            in_=ot[g * blk: g * blk + Cout, 0:nr, 0:W])
                if sid is not None:
                    emit_stats_reduce(sid, stile, scnt)

            # -------------------------------------------------------- conv0
            def emit_conv0(P):
                Cout, a, b, lvl = TENS['h0']
                W = 512
                nch, blk = P['nch'], P['blk']
                O0, O1 = LVL_OWN[0]
                owned, halo = slot_plan(a, b, O0, O1, nch, RPC)
                (tix, poff, kn, krow0, col, co) = WPLAN['conv0'][0][0]

                def do_chunk(xt, g, r0, r1):
                    nr = r1 - r0
                    for dy in range(3):
                        src = bass.AP(tensor=x_in, offset=(r0 - 1 + dy) * 514,
                                      ap=[[1, 3], [514, nr], [1, 512]])
                        nc.sync.dma_start(out=xt[blk * g + 3 * dy: blk * g + 3 * dy + 3, 0:nr, :],
                                          in_=src)

                for chunks in owned:
                    xt = inb.tile([128, RPC, 512], F32, tag="xin")
                    ot = outb.tile([128, RPC, W], F32, tag="ot")
                    for g, (r0, r1) in enumerate(chunks):
                        do_chunk(xt, g, r0, r1)
                    for j in range(RPC):
                        pt = psp.tile([128, 512], F32, tag="pt")
                        for g in range(nch):
                            nc.tensor.matmul(out=pt[g * blk: g * blk + Cout, 0:W],
                                             lhsT=wsb[g * blk: g * blk + 9, col: col + 16],
                                             rhs=xt[g * blk: g * blk + 9, j, :],
                                             start=True, stop=True,
                                             tile_position=(g * blk, g * blk))
                        nc.scalar.copy(out=ot[:, j, :], in_=pt[:, 0:W])
                    for g, (r0, r1) in enumerate(chunks):
                        nc.sync.dma_start(out=dram_ap('h0', 0, Cout, r0, r1, interior=True),
                                          in_=ot[g * blk: g * blk + Cout, :, 0:W])
                for chunks in halo:
                    xt = inb.tile([128, RPC, 512], F32, tag="xin")
                    ot = outb.tile([128, RPC, W], F32, tag="ot")
                    for g, rr in enumerate(chunks):
                        if rr is None:
                            continue
                        r0, r1 = rr
                        nr = r1 - r0
                        do_chunk(xt, g, r0, r1)
                        for j in range(nr):
                            pt = psp.tile([128, 512], F32, tag="pt")
                            nc.tensor.matmul(out=pt[g * blk: g * blk + Cout, 0:W],
                                             lhsT=wsb[g * blk: g * blk + 9, col: col + 16],
                                             rhs=xt[g * blk: g * blk + 9, j, :],
                                             start=True, stop=True,
                                             tile_position=(g * blk, g * blk))
                            nc.scalar.copy(out=ot[g * blk: g * blk + Cout, j, :],
                                           in_=pt[g * blk: g * blk + Cout, 0:W])
                        nc.sync.dma_start(out=dram_ap('h0', 0, Cout, r0, r1, interior=True),
                                          in_=ot[g * blk: g * blk + Cout, 0:nr, 0:W])

            # --------------------------------------------------------- comb
            def emit_comb(P):
                name = P['name']
                Cout, a, b, lvl = TENS[P['out']]
                W = LVL_W[lvl]
                WP = W + 2
                O0, O1 = LVL_OWN[lvl]
                nch, blk = P['nch'], P['blk']
                rps = max(1, 512 // W)
                owned, halo = slot_plan(a, b, O0, O1, nch, RPC)
                sid = P.get('stats')
                has_ws = P['wkey'] is not None
                taps = WPLAN[name] if has_ws else None
                two = any(s[3] == 1 for s in P['srcs'])
                last_tix = P['srcs'][-1][3]
                stile = None
                scnt = 0
                if sid is not None:
                    nslots = (O1 - O0) // nch // rps
                    stile = statsp.tile([128, nslots, 6], F32, tag="st")

                def slot_body(chunklist, uniform):
                    nonlocal scnt
                    yt = inb.tile([128, RPC, WP], F32, tag="yt")
                    xt0 = inb.tile([128, RPC, WP], F32, tag="xin")
                    tiles = [xt0]
                    if two:
                        xt1 = inb.tile([128, RPC, WP], F32, tag="xin2")
                        tiles.append(xt1)
                    ot = outb.tile([128, RPC, W], F32, tag="ot")
                    for g, rr in enumerate(chunklist):
                        if rr is None:
                            continue
                        r0, r1 = rr
                        nr = r1 - r0
                        nc.sync.dma_start(out=yt[g * blk: g * blk + Cout, 0:nr, :],
                                          in_=dram_ap(P['y2'], 0, Cout, r0, r1))
                        load_band(P, tiles, g, r0, r1, 0)
                    full = slice(None) if uniform else None
                    for g, rr in enumerate(chunklist):
                        if rr is None:
                            continue
                        r0, r1 = rr
                        nr = r1 - r0
                        sl = slice(0, 128) if uniform and g == 0 else slice(g * blk, (g + 1) * blk)
                        if uniform and g > 0:
                            continue
                        nc.vector.tensor_scalar(
                            out=yt[sl, 0:nr, 1: W + 1], in0=yt[sl, 0:nr, 1: W + 1],
                            scalar1=abn[sl, P['ybn'], 0:1], scalar2=abn[sl, P['ybn'], 1:2],
                            op0=ALU.mult, op1=ALU.add)
                    for g, rr in enumerate(chunklist):
                        if rr is None:
                            continue
                        r0, r1 = rr
                        nr = r1 - r0
                        if uniform and g > 0:
                            continue
                        uni = uniform
                        j = 0
                        while j < nr:
                            nrs = min(rps, nr - j)
                            gs = range(nch) if uni else [g]
                            if has_ws:
                                pt = psp.tile([128, 512], F32, tag="pt")
                                for gg in gs:
                                    first = True
                                    for (tix, poff, kn, krow0, col, co) in taps[0]:
                                        nc.tensor.matmul(
                                            out=pt[gg * blk: gg * blk + Cout, 0: nrs * W],
                                            lhsT=wsb[gg * blk + poff: gg * blk + poff + kn, col: col + co],
                                            rhs=tiles[tix][gg * blk + poff: gg * blk + poff + kn,
                                                           j: j + nrs, 1: W + 1],
                                            start=first, stop=(tix == last_tix),
                                            tile_position=(gg * blk, gg * blk) if nch > 1 else None)
                                        first = False
                                psl = slice(0, 128) if uni else slice(g * blk, (g + 1) * blk)
                                nc.vector.tensor_tensor(
                                    out=ot[psl, j: j + nrs, :],
                                    in0=yt[psl, j: j + nrs, 1: W + 1],
                                    in1=pt[psl, 0: nrs * W].rearrange("p (r w) -> p r w", r=nrs),
                                    op=ALU.add)
                            else:
                                psl = slice(0, 128) if uni else slice(g * blk, (g + 1) * blk)
                                nc.vector.tensor_tensor(
                                    out=ot[psl, j: j + nrs, :],
                                    in0=yt[psl, j: j + nrs, 1: W + 1],
                                    in1=tiles[0][psl, j: j + nrs, 1: W + 1],
                                    op=ALU.add)
                            j += nrs
                        psl = slice(0, 128) if uni else slice(g * blk, (g + 1) * blk)
                        nc.scalar.activation(out=ot[psl, 0:nr, :], in_=ot[psl, 0:nr, :],
                                             func=AF.Relu)
                        if sid is not None and uni:
                            j = 0
                            while j < nr:
                                nrs = min(rps, nr - j)
                                nc.vector.bn_stats(
                                    out=stile[:, scnt, :],
                                    in_=ot[:, j: j + nrs, :].rearrange("p r w -> p (r w)"))
                                scnt += 1
                                j += nrs
                    for g, rr in enumerate(chunklist):
                        if rr is None:
                            continue
                        r0, r1 = rr
                        nc.sync.dma_start(
                            out=dram_ap(P['out'], 0, Cout, r0, r1, interior=True),
                            in_=ot[g * blk: g * blk + Cout, 0: r1 - r0, 0:W])

                for chunks in owned:
                    slot_body(chunks, True)
                for chunks in halo:
                    slot_body(chunks, False)
                if sid is not None:
                    emit_stats_reduce(sid, stile, scnt)

            # --------------------------------------------------------- down
            def emit_down(P):
                name = P['name']
                Cout, a, b, lvl = TENS[P['out']]
                W = LVL_W[lvl]
                Wi = LVL_W[lvl - 1]
                WPi = Wi + 2
                O0, O1 = LVL_OWN[lvl]
                nch, blk = P['nch'], P['blk']
                rpc = 4
                rps = min(max(1, 512 // W), rpc)
                owned, halo = slot_plan(a, b, O0, O1, nch, rpc)
                taps = WPLAN[name]
                (tname, c0, c1, _, poff) = P['srcs'][0]

                def chunk_mms(xt, pt, g, j, nrs):
                    for t in range(4):
                        ta, tb = t // 2, t % 2
                        (tix_, poff_, kn, krow0, col, co) = taps[t][0]
                        nc.tensor.matmul(
                            out=pt[g * blk: g * blk + Cout, 0: nrs * W],
                            lhsT=wsb[g * blk: g * blk + kn, col: col + co],
                            rhs=xt[g * blk: g * blk + kn,
                                   2 * j + ta: 2 * j + ta + 2 * nrs: 2,
                                   tb + 1: tb + 1 + 2 * W: 2],
                            start=(t == 0), stop=(t == 3),
                            tile_position=(g * blk, g * blk) if nch > 1 else None)

                for chunks in owned:
                    xt = inb.tile([128, 2 * rpc + 1, WPi], F32, tag="xin")
                    ot = outb.tile([128, rpc, W], F32, tag="dot")
                    for g, (r0, r1) in enumerate(chunks):
                        nc.sync.dma_start(
                            out=xt[g * blk + poff: g * blk + poff + (c1 - c0), 0: 2 * rpc, :],
                            in_=dram_ap(tname, c0, c1, 2 * r0, 2 * r1))
                    for j in range(0, rpc, rps):
                        pt = psp.tile([128, 512], F32, tag="pt")
                        for g in range(nch):
                            chunk_mms(xt, pt, g, j, rps)
                        nc.scalar.copy(out=ot[:, j: j + rps, :], in_=pt[:, 0: rps * W])
                    for g, (r0, r1) in enumerate(chunks):
                        nc.sync.dma_start(out=dram_ap(P['out'], 0, Cout, r0, r1, interior=True),
                                          in_=ot[g * blk: g * blk + Cout, :, 0:W])
                for chunks in halo:
                    xt = inb.tile([128, 2 * rpc + 1, WPi], F32, tag="xin")
                    ot = outb.tile([128, rpc, W], F32, tag="dot")
                    for g, rr in enumerate(chunks):
                        if rr is None:
                            continue
                        r0, r1 = rr
                        nr = r1 - r0
                        nc.sync.dma_start(
                            out=xt[g * blk + poff: g * blk + poff + (c1 - c0), 0: 2 * nr, :],
                            in_=dram_ap(tname, c0, c1, 2 * r0, 2 * r1))
                        j = 0
                        while j < nr:
                            nrs = min(rps, nr - j)
                            pt = psp.tile([128, 512], F32, tag="pt")
                            chunk_mms(xt, pt, g, j, nrs)
                            nc.scalar.copy(out=ot[g * blk: g * blk + Cout, j: j + nrs, :],
                                           in_=pt[g * blk: g * blk + Cout, 0: nrs * W])
                            j += nrs
                        nc.sync.dma_start(out=dram_ap(P['out'], 0, Cout, r0, r1, interior=True),
                                          in_=ot[g * blk: g * blk + Cout, 0:nr, 0:W])

            # ----------------------------------------------------------- up
            def emit_up(P):
                name = P['name']
                Cout, a, b, lvl = TENS[P['out']]
                W = LVL_W[lvl]
                Wi = LVL_W[lvl + 1]
                WPi = Wi + 2
                src = P['srcs'][0]
                Cin = src[2] - src[1]
                ia, ib_ = a // 2, (b + 1) // 2
                nch, blk = P['nch'], P['blk']
                rpsi = max(1, 512 // Wi)
                taps = WPLAN[name]
                rpci = 4
                T = -(-(ib_ - ia) // nch)
                ch_rng = [(ia + g * T, min(ia + (g + 1) * T, ib_)) for g in range(nch)]
                nslot = -(-T // rpci)
                for q in range(nslot):
                    xt = inb.tile([128, rpci, WPi], F32, tag="xin")
                    ot = outb.tile([128, 2 * rpci + 1, W + 1], F32, tag="ot")
                    act = []
                    for g in range(nch):
                        r0 = ch_rng[g][0] + q * rpci
                        r1 = min(r0 + rpci, ch_rng[g][1])
                        if r0 < r1:
                            act.append((g, r0, r1))
                    for g, r0, r1 in act:
                        nc.sync.dma_start(out=xt[g * blk: g * blk + Cin, 0: r1 - r0, :],
                                          in_=dram_ap(src[0], src[1], src[2], r0, r1))
                    nrmax = max(r1 - r0 for g, r0, r1 in act)
                    j = 0
                    while j < nrmax:
                        nrs = min(rpsi, nrmax - j)
                        for t in range(4):
                            ta, tb = t // 2, t % 2
                            (tix, poff, kn, krow0, col, co) = taps[t][0]
                            pt = psp.tile([128, 512], F32, tag="pt")
                            for g, r0, r1 in act:
                                if j >= r1 - r0:
                                    continue
                                nc.tensor.matmul(
                                    out=pt[g * blk: g * blk + Cout, 0: nrs * Wi],
                                    lhsT=wsb[g * blk: g * blk + kn, col: col + co],
                                    rhs=xt[g * blk: g * blk + kn, j: j + nrs, 1: Wi + 1],
                                    start=True, stop=True,
                                    tile_position=(g * blk, g * blk) if nch > 1 else None)
                            ro = 2 * j + ta
                            nc.scalar.copy(
                                out=ot[:, ro: ro + 2 * nrs: 2, tb: tb + 2 * Wi: 2],
                                in_=pt[:, 0: nrs * Wi])
                        j += nrs
                    for g, r0, r1 in act:
                        oa, ob = max(2 * r0, a), min(2 * r1, b)
                        nc.sync.dma_start(
                            out=dram_ap(P['out'], 0, Cout, oa, ob, interior=True),
                            in_=ot[g * blk: g * blk + Cout, oa - 2 * r0: ob - 2 * r0, 0:W])

            # --------------------------------------------------------- pool
            def emit_pool(P):
                Cs, a, b, lvl = TENS['f2o']
                W = 512
                nch, blk = P['nch'], P['blk']
                O0, O1 = LVL_OWN[0]
                owned, halo = slot_plan(a, b, O0, O1, nch, RPC)
                assert not halo or all(all(r is None for r in h) for h in halo)
                nb = len(owned)
                acc = statsp.tile([128, max(nb, 1)], F32, tag="acc")
                for i, chunks in enumerate(owned):
                    xt = inb.tile([128, RPC, W + 2], F32, tag="yt")
                    ot = outb.tile([128, RPC, W], F32, tag="ot")
                    for g, (r0, r1) in enumerate(chunks):
                        nc.sync.dma_start(out=xt[g * blk: g * blk + Cs, :, :],
                                          in_=dram_ap('f2o', 0, Cs, r0, r1))
                    nc.scalar.activation(
                        out=ot[:, :, :], in_=xt[:, :, 1: W + 1], func=AF.Relu,
                        scale=abn[:, P['prep'], 0:1], bias=abn[:, P['prep'], 1:2])
                    nc.vector.tensor_reduce(
                        out=acc[:, i: i + 1],
                        in_=ot[:, :, :].rearrange("p r w -> p (r w)"),
                        axis=mybir.AxisListType.X, op=ALU.add)
                accs = misc.tile([128, 1], F32, tag="accs")
                nc.vector.tensor_reduce(out=accs, in_=acc[:, 0:nb],
                                        axis=mybir.AxisListType.X, op=ALU.add)
                nc.sync.dma_start(out=pooled[:, :], in_=accs)

            for P in PASSES:
                with nc.named_scope(P['name']):
                    {'conv0': emit_conv0, 'conv': emit_conv, 'comb': emit_comb,
                     'down': emit_down, 'up': emit_up, 'pool': emit_pool}[P['kind']](P)

    nc.compile()
    return nc


# ------------------------------------------------------------ host driver
_NC_CACHE = {}


def _get_nc():
    if 'nc' not in _NC_CACHE:
        _NC_CACHE['nc'] = build_bass()
    return _NC_CACHE['nc']


def _slab_for_core(x, core):
    """x: [4,512,512,1] -> padded slab [352, 514] for this core."""
    b, half = core // 2, core % 2
    img = np.asarray(x[b, :, :, 0], np.float32)
    slab = np.zeros((SLAB, 514), np.float32)
    r0 = half * 256 - HALO
    lo = max(0, -r0)
    hi = min(SLAB, H - r0)
    slab[lo:hi, 1:513] = img[r0 + lo: r0 + hi]
    return slab


def kernel(x, params):
    from concourse.bass_utils import run_bass_kernel_spmd

    x = np.asarray(x, np.float32)
    nc = _get_nc()
    wpack = pack_weights(params)
    bnpack = pack_bn(params)
    in_maps = [{"x": _slab_for_core(x, c), "w": wpack, "bn": bnpack} for c in range(8)]
    import os
    trace = bool(int(os.environ.get("KERNEL_TRACE", "0")))
    res = run_bass_kernel_spmd(nc, in_maps, core_ids=list(range(8)), trace=trace)
    _NC_CACHE['last_result'] = res

    lin_w = np.asarray(params['lin_w'], np.float32)
    lin_b = np.asarray(params['lin_b'], np.float32)
    hfeat = np.zeros((B, 16), np.float32)
    for c in range(8):
        p = res.results[c]["pooled"][:, 0].reshape(4, 32)[:, :16].sum(0)
        hfeat[c // 2] += p
    hfeat /= float(H * WIDTH)
    return (hfeat @ lin_w + lin_b).astype(np.float32)
